# revision 1
# baseline (speedup 1.0000x reference)
"""Cuboid self-attention layer (pre-LN, shifted-window) on 8 Trainium2 cores.

Strategy: the (batch, cuboid) axis gives 2*512 = 1024 independent attention
problems of seq len 98; each core processes 128 of them with an identical
instruction stream. Scores are computed transposed (S^T[j,(h,i)]) so no
on-chip transposes are needed in the attention inner loop; softmax runs
without max-subtraction (scores are bounded by construction) and its
normalization is applied to the attention *output* via a per-head
denominator block computed with a ones-stationary matmul. The shift-window
mask is rank<=8 (region-indicator G @ G.T): a standalone matmul produces the
0/1 mask in PSUM and one vector multiply applies it after exp — only for the
43 masked slots per core (pairs are re-scheduled host-side so every core
gets 85 unmasked + 43 masked slots; leftover unmasked pairs ride in masked
slots with an all-ones G). LayerNorm affine and all biases are folded into
the weight matrices / per-partition bias adds host-side.
"""

import os
import numpy as np

B, T, H, W, C = 2, 16, 56, 56, 128
CB_T, CB_H, CB_W = 2, 7, 7
SH_T, SH_H, SH_W = 1, 3, 3
NUM_HEADS, HD = 4, 32
VOL = CB_T * CB_H * CB_W            # 98
NBT, NBH, NBW = T // CB_T, H // CB_H, W // CB_W    # 8, 8, 8
NCUB = NBT * NBH * NBW              # 512
NPAIRS = B * NCUB                   # 1024
NCORES = 8
PPC = NPAIRS // NCORES              # 128 pairs per core
UNM = 85                            # unmasked slots per core
MSK = PPC - UNM                     # masked slots per core (43)
LN_EPS = 1e-5
SCALE = HD ** -0.5


def _cuboid_reorder_np(data):
    # (B,T,H,W,C) -> (B, NCUB, VOL, C)
    b = data.reshape(B, NBT, CB_T, NBH, CB_H, NBW, CB_W, C)
    b = b.transpose(0, 1, 3, 5, 2, 4, 6, 7)
    return b.reshape(B, NCUB, VOL, C)


def _cuboid_reorder_reverse_np(data):
    # (B, NCUB, VOL, C) -> (B,T,H,W,C)
    b = data.reshape(B, NBT, NBH, NBW, CB_T, CB_H, CB_W, C)
    b = b.transpose(0, 1, 4, 2, 5, 3, 6, 7)
    return b.reshape(B, T, H, W, C)


def _region_ids():
    """sm region id per (cuboid n, token l) on the unrolled grid: [NCUB, VOL]."""
    def reg(idx, n, cs, ss):
        return np.where(idx < n - cs, 0, np.where(idx < n - ss, 1, 2))

    rt = reg(np.arange(T), T, CB_T, SH_T)
    rh = reg(np.arange(H), H, CB_H, SH_H)
    rw = reg(np.arange(W), W, CB_W, SH_W)
    sm = (rt[:, None, None] * 9 + rh[None, :, None] * 3 + rw[None, None, :])
    b = sm.reshape(1, NBT, CB_T, NBH, CB_H, NBW, CB_W, 1)
    b = b.transpose(0, 1, 3, 5, 2, 4, 6, 7).reshape(NCUB, VOL)
    return b.astype(np.int32)


def _g_matrices():
    """Per-cuboid 0/1 region indicators, [NCUB, 8, VOL] f32."""
    sm = _region_ids()
    g = np.zeros((NCUB, 8, VOL), np.float32)
    for n in range(NCUB):
        vals = np.unique(sm[n])
        assert len(vals) <= 8
        for r, v in enumerate(vals):
            g[n, r, :] = (sm[n] == v)
    return g


def _schedule():
    """Assign global pairs to (core, slot): slots [0,UNM) unmasked, rest masked.

    Returns perm [NPAIRS] = global pair index for flattened (core, slot).
    """
    n = np.arange(NCUB)
    flag = (n // (NBH * NBW) == NBT - 1) | ((n // NBW) % NBH == NBH - 1) \
        | (n % NBW == NBW - 1)
    flag = np.concatenate([flag, flag])            # both batches
    unmasked = np.where(~flag)[0]
    masked = np.where(flag)[0]
    assert len(unmasked) >= UNM * NCORES
    fillers = unmasked[UNM * NCORES:]
    rem = np.concatenate([masked, fillers])
    assert len(rem) == MSK * NCORES
    perm = np.empty(NPAIRS, np.int64)
    for c in range(NCORES):
        perm[c * PPC:c * PPC + UNM] = unmasked[c * UNM:(c + 1) * UNM]
        perm[c * PPC + UNM:(c + 1) * PPC] = rem[c * MSK:(c + 1) * MSK]
    return perm


_G_CACHE = None
_PERM_CACHE = None


def _host_prep(x, gamma, beta, w_qkv, w_proj, b_proj):
    global _G_CACHE, _PERM_CACHE
    x = np.asarray(x, np.float32)
    gamma = np.asarray(gamma, np.float32)
    beta = np.asarray(beta, np.float32)
    w_qkv = np.asarray(w_qkv, np.float32)
    w_proj = np.asarray(w_proj, np.float32)
    b_proj = np.asarray(b_proj, np.float32)

    shifted = np.roll(x, shift=(-SH_T, -SH_H, -SH_W), axis=(1, 2, 3))
    xr = _cuboid_reorder_np(shifted).reshape(NPAIRS, VOL, C)

    if _G_CACHE is None:
        _G_CACHE = _g_matrices()
    if _PERM_CACHE is None:
        _PERM_CACHE = _schedule()
    perm = _PERM_CACHE

    xr_sched = xr[perm]                            # [NPAIRS, VOL, C]
    gfull = np.concatenate([_G_CACHE, _G_CACHE])   # [NPAIRS, 8, VOL]
    g_sched = gfull[perm].reshape(NCORES, PPC, 8, VOL)[:, UNM:]  # [NC, MSK, 8, VOL]

    wg = gamma[:, None] * w_qkv
    wq = np.ascontiguousarray(wg[:, 0:C] * SCALE)
    wk = np.ascontiguousarray(wg[:, C:2 * C])
    wv = np.ascontiguousarray(wg[:, 2 * C:3 * C])
    bq = (beta @ w_qkv[:, 0:C]) * SCALE
    bk = beta @ w_qkv[:, C:2 * C]
    bv = beta @ w_qkv[:, 2 * C:3 * C]
    bfin = b_proj + bv @ w_proj

    consts = dict(
        wq=wq, wk=wk, wv=wv, wp=w_proj,
        bq=bq.reshape(C, 1), bk=bk.reshape(C, 1), bfin=bfin.reshape(C, 1),
        i98=np.eye(VOL, dtype=np.float32),
    )
    return xr_sched, g_sched, perm, consts


def _host_post(outs, perm):
    # outs: list of NCORES arrays [PPC, C, VOL], in scheduled order
    o = np.concatenate(outs, axis=0)               # [NPAIRS, C, VOL]
    inv = np.empty_like(perm)
    inv[perm] = np.arange(NPAIRS)
    o = o[inv]
    o = np.ascontiguousarray(o.transpose(0, 2, 1)).reshape(B, NCUB, VOL, C)
    o = _cuboid_reorder_reverse_np(o)
    return np.roll(o, shift=(SH_T, SH_H, SH_W), axis=(1, 2, 3))


_NC_CACHE = {}


def _build_program(fused_qk_copy, ppc=PPC, unm=UNM, do_compile=True):
    """Build and compile the per-core Bass program (identical on all cores)."""
    from contextlib import ExitStack
    import concourse.bass as bass
    import concourse.bacc as bacc
    import concourse.tile as tile
    import concourse.mybir as mybir

    key = (fused_qk_copy, ppc, unm)
    if key in _NC_CACHE:
        return _NC_CACHE[key]
    msk = ppc - unm

    f32 = mybir.dt.float32
    AF = mybir.ActivationFunctionType
    ALU = mybir.AluOpType

    nc = bacc.Bacc()
    xr_d = nc.declare_dram_parameter("xr", [ppc, VOL, C], f32, isOutput=False)
    g_d = nc.declare_dram_parameter("g", [max(msk, 1), 8, VOL], f32, isOutput=False)
    wq_d = nc.declare_dram_parameter("wq", [C, C], f32, isOutput=False)
    wk_d = nc.declare_dram_parameter("wk", [C, C], f32, isOutput=False)
    wv_d = nc.declare_dram_parameter("wv", [C, C], f32, isOutput=False)
    wp_d = nc.declare_dram_parameter("wp", [C, C], f32, isOutput=False)
    bq_d = nc.declare_dram_parameter("bq", [C, 1], f32, isOutput=False)
    bk_d = nc.declare_dram_parameter("bk", [C, 1], f32, isOutput=False)
    bfin_d = nc.declare_dram_parameter("bfin", [C, 1], f32, isOutput=False)
    i98_d = nc.declare_dram_parameter("i98", [VOL, VOL], f32, isOutput=False)
    out_d = nc.declare_dram_parameter("out", [ppc, C, VOL], f32, isOutput=True)

    with tile.TileContext(nc) as tc, ExitStack() as ctx:
        const = ctx.enter_context(tc.tile_pool(name="const", bufs=1))
        work = ctx.enter_context(tc.tile_pool(name="work", bufs=3))
        ps = ctx.enter_context(tc.tile_pool(name="ps", bufs=2, space="PSUM"))

        wq_t = const.tile([C, C], f32)
        nc.gpsimd.dma_start(out=wq_t[:], in_=wq_d[:])
        wk_t = const.tile([C, C], f32)
        nc.gpsimd.dma_start(out=wk_t[:], in_=wk_d[:])
        wv_t = const.tile([C, C], f32)
        nc.gpsimd.dma_start(out=wv_t[:], in_=wv_d[:])
        wp_t = const.tile([C, C], f32)
        nc.gpsimd.dma_start(out=wp_t[:], in_=wp_d[:])
        bq_t = const.tile([C, 1], f32)
        nc.gpsimd.dma_start(out=bq_t[:], in_=bq_d[:])
        bk_t = const.tile([C, 1], f32)
        nc.gpsimd.dma_start(out=bk_t[:], in_=bk_d[:])
        bfin_t = const.tile([C, 1], f32)
        nc.gpsimd.dma_start(out=bfin_t[:], in_=bfin_d[:])
        i98_t = const.tile([VOL, VOL], f32)
        nc.gpsimd.dma_start(out=i98_t[:], in_=i98_d[:])
        eps_t = const.tile([VOL, 1], f32)
        nc.vector.memset(eps_t[:], LN_EPS)
        ones32_t = const.tile([VOL, HD], f32)
        nc.vector.memset(ones32_t[:], 1.0)

        psq = ctx.enter_context(tc.tile_pool(name="psq", bufs=1, space="PSUM"))
        psk = ctx.enter_context(tc.tile_pool(name="psk", bufs=1, space="PSUM"))
        # persistent block-diagonal q bank: zeroed once, only the 4 diagonal
        # blocks are rewritten each pair
        bankQ = psq.tile([128, 512], f32)
        nc.vector.memset(bankQ[:], 0.0)

        for p in range(ppc):
            masked = p >= unm
            # ---- load
            x_t = work.tile([VOL, C], f32)
            nc.gpsimd.dma_start(out=x_t[:], in_=xr_d[p])

            # ---- layernorm (standardize only; affine folded into weights)
            st = work.tile([VOL, 6], f32)
            nc.vector.bn_stats(out=st[:], in_=x_t[:])
            mv = work.tile([VOL, 2], f32)
            nc.vector.bn_aggr(out=mv[:], in_=st[:])
            sd = work.tile([VOL, 1], f32)
            nc.scalar.activation(out=sd[:], in_=mv[:, 1:2], func=AF.Sqrt,
                                 bias=eps_t[:], scale=1.0)
            rstd = work.tile([VOL, 1], f32)
            nc.vector.reciprocal(out=rstd[:], in_=sd[:])
            xn = work.tile([VOL, C], f32)
            nc.gpsimd.tensor_scalar(xn[:], x_t[:], mv[:, 0:1], rstd[:],
                                    ALU.subtract, ALU.mult)

            # ---- transpose xn -> [C, VOL]; mask matmul shares bankC
            bankC = ps.tile([128, 512], f32, tag="bankC")
            nc.tensor.transpose(bankC[:, 0:VOL], xn[:], i98_t[:])
            xnT = work.tile([C, VOL], f32)
            nc.scalar.copy(out=xnT[:], in_=bankC[:, 0:VOL])
            if masked:
                g_t = work.tile([8, VOL], f32)
                nc.gpsimd.dma_start(out=g_t[:], in_=g_d[p - unm])
                nc.tensor.matmul(bankC[0:VOL, VOL:2 * VOL], g_t[:], g_t[:],
                                 start=True, stop=True)

            # ---- qkv projections. q is produced block-diagonal in PSUM
            # (head h at rows 32h, cols 98h of a once-zeroed dedicated bank)
            # so all 4 heads' scores come from ONE K=128 matmul — fp32
            # matmuls that switch PE row-strips hang this hardware.
            for hh in range(NUM_HEADS):
                nc.tensor.matmul(
                    bankQ[hh * HD:(hh + 1) * HD, hh * VOL:(hh + 1) * VOL],
                    wq_t[:, hh * HD:(hh + 1) * HD], xnT[:],
                    start=True, stop=True, skip_group_check=True,
                    tile_position=(0, hh * HD))
            bankK = psk.tile([128, 512], f32)
            nc.tensor.matmul(bankK[:, 0:VOL], wk_t[:], xnT[:], start=True, stop=True)
            qbd = work.tile([C, NUM_HEADS * VOL], f32)
            nc.vector.tensor_copy(out=qbd[:], in_=bankQ[:, 0:NUM_HEADS * VOL])
            if not fused_qk_copy:
                for hh in range(NUM_HEADS):
                    nc.vector.tensor_scalar_add(
                        qbd[hh * HD:(hh + 1) * HD, hh * VOL:(hh + 1) * VOL],
                        qbd[hh * HD:(hh + 1) * HD, hh * VOL:(hh + 1) * VOL],
                        bq_t[hh * HD:(hh + 1) * HD, :])
            kT = work.tile([C, VOL], f32)
            if fused_qk_copy:
                nc.scalar.copy(out=kT[:], in_=bankK[:, 0:VOL])
            else:
                nc.scalar.add(out=kT[:], in_=bankK[:, 0:VOL], add=bk_t[:])

            bankB = ps.tile([128, 512], f32, tag="bankB")
            nc.tensor.matmul(bankB[0:VOL, 3 * VOL:3 * VOL + C], xnT[:], wv_t[:],
                             start=True, stop=True)
            v_sb = work.tile([VOL, C], f32)
            nc.scalar.copy(out=v_sb[:], in_=bankB[0:VOL, 3 * VOL:3 * VOL + C])

            # ---- scores S^T[j, (h,i)] for all heads in one matmul
            bankA = ps.tile([128, 512], f32, tag="bankA")
            nc.tensor.matmul(bankA[0:VOL, 0:NUM_HEADS * VOL], kT[:], qbd[:],
                             start=True, stop=True)

            # ---- exp (no max subtraction; scores bounded), then 0/1 mask
            pexp = work.tile([VOL, NUM_HEADS * VOL], f32)
            nc.scalar.activation(out=pexp[:], in_=bankA[0:VOL, 0:NUM_HEADS * VOL],
                                 func=AF.Exp)
            if masked:
                mrep = bankC[0:VOL, VOL:2 * VOL]
                m_ap = bass.AP(tensor=mrep.tensor, offset=mrep.offset,
                               ap=[mrep.ap[0], [0, NUM_HEADS], mrep.ap[1]])
                pe3 = pexp[:].rearrange("p (h w) -> p h w", h=NUM_HEADS)
                nc.vector.tensor_mul(out=pe3, in0=pe3, in1=m_ap)

            # ---- per-head denominators, replicated across each head's 32 rows;
            # reciprocal of the block IS the normalization matrix R.
            for hh in range(NUM_HEADS):
                nc.tensor.matmul(
                    bankB[hh * HD:(hh + 1) * HD, VOL:2 * VOL],
                    ones32_t[:], pexp[:, hh * VOL:(hh + 1) * VOL],
                    start=True, stop=True, skip_group_check=True,
                    tile_position=(0, hh * HD))
            r_sb = work.tile([C, VOL], f32)
            nc.vector.reciprocal(out=r_sb[:], in_=bankB[:, VOL:2 * VOL])

            # ---- attention output (transposed, unnormalized)
            for hh in range(NUM_HEADS):
                nc.tensor.matmul(
                    bankB[hh * HD:(hh + 1) * HD, 0:VOL],
                    v_sb[:, hh * HD:(hh + 1) * HD],
                    pexp[:, hh * VOL:(hh + 1) * VOL],
                    start=True, stop=True, skip_group_check=True,
                    tile_position=(0, hh * HD))

            # ---- normalize + project + final bias
            on_sb = work.tile([C, VOL], f32)
            nc.vector.tensor_mul(out=on_sb[:], in0=bankB[:, 0:VOL], in1=r_sb[:])
            nc.tensor.matmul(bankB[:, 2 * VOL:3 * VOL], wp_t[:], on_sb[:],
                             start=True, stop=True)
            out_sb = work.tile([C, VOL], f32)
            nc.vector.tensor_scalar_add(out_sb[:], bankB[:, 2 * VOL:3 * VOL],
                                        bfin_t[:])
            nc.gpsimd.dma_start(out=out_d[p], in_=out_sb[:])

    if do_compile:
        nc.compile()
    _NC_CACHE[key] = nc
    return nc


def kernel(x, gamma, beta, w_qkv, w_proj, b_proj):
    from concourse.bass_utils import run_bass_kernel_spmd

    xr, g, perm, consts = _host_prep(x, gamma, beta, w_qkv, w_proj, b_proj)
    fused = not np.any(np.asarray(beta, np.float32))
    nc = _build_program(fused)

    in_maps = []
    for c in range(NCORES):
        m = dict(consts)
        m["xr"] = np.ascontiguousarray(xr[c * PPC:(c + 1) * PPC])
        m["g"] = np.ascontiguousarray(g[c])
        in_maps.append(m)

    res = run_bass_kernel_spmd(nc, in_maps, list(range(NCORES)),
                               trace=os.environ.get("KERNEL_TRACE", "") == "1")
    global LAST_EXEC_NS, LAST_TRACE
    LAST_EXEC_NS = res.exec_time_ns
    LAST_TRACE = res.instructions_and_trace
    outs = [res.results[i]["out"] for i in range(NCORES)]
    return _host_post(outs, perm).astype(np.float32)


LAST_EXEC_NS = None
LAST_TRACE = None

